# revision 1
# baseline (speedup 1.0000x reference)
"""GRU policy kernel for Trainium2 (8 NeuronCores, data-parallel over batch).

Problem: nn_GRUPolicy — B=2048, T=512, V=4, E=64, H=128.

  xe = emb[x]                          # [B,T,E]
  gi = xe @ W_ih.T + b_ih              # [B,T,3H]
  scan over t: GRU cell (PyTorch gate order r,z,n)
  logits = h_T @ W_fc.T + b_fc         # [B,V]

Key algebraic facts exploited:
  * V=4 so the whole input-side projection collapses into a [4, 3H]
    lookup table giTab = emb @ W_ih.T + b_ih (+ b_hh folded in); per
    step it is realized on-device as a K=4 one-hot matmul accumulated
    straight into the same PSUM region as the recurrence matmul.
  * Everything is kept transposed ([H, batch] on 128 partitions) so the
    recurrence never needs a transpose.
  * h' = (1-z)*n + z*h  ==  z*h - (z-1)*n  -> one GPSIMD mult (p=z*h,
    off the critical path), one fused scalar_tensor_tensor
    (q=(z-1)*n) and one subtract.
  * b_hh_n rides for free inside the fused u = (ghn + b_hh_n) * r.

Sharding: batch 2048 -> 8 cores x 256; each core runs 2 independent
128-column chains, emitted interleaved by op-kind, so the serial
per-step dependency chain of one chain overlaps with engine work of
the other. Measured ~1.2-1.5 ms on 8 tunneled trn2 cores (~2.4 us per
GRU step), rel err ~1.4e-3 vs the fp32 reference.
"""

import sys

import numpy as np

for _p in ("/opt/trn_rl_repo",):
    if _p not in sys.path:
        sys.path.insert(0, _p)

from concourse import bacc, bass, mybir, tile  # noqa: E402
from concourse.bass_utils import run_bass_kernel_spmd  # noqa: E402

F16 = mybir.dt.float16
F32 = mybir.dt.float32
AF = mybir.ActivationFunctionType
OP = mybir.AluOpType

B, T, V, E, H = 2048, 512, 4, 64, 128
N_CORES = 8
BS = B // N_CORES          # 256 batch rows per core
NCH = 2                    # independent chains per core
USE_GPS = True             # p = z*h on GPSIMD
WBUFS = 3                  # work pool depth
W = BS // NCH              # 128 batch columns per chain
CHUNK = 64                 # time steps per one-hot DMA chunk


def build_nc(t_steps: int = T, dump_h: bool = False, reps: int = 1, nch: int = NCH, use_gps: bool = USE_GPS, wbufs: int = WBUFS) -> bass.Bass:
    nc = bacc.Bacc(None)

    oh_d = nc.dram_tensor("oh", [V, t_steps * BS], F16, kind="ExternalInput")
    wt_d = nc.dram_tensor("WT", [H, 3 * H], F16, kind="ExternalInput")
    gi_d = nc.dram_tensor("giT", [V, 3 * H], F16, kind="ExternalInput")
    wf_d = nc.dram_tensor("WfcT", [H, V], F16, kind="ExternalInput")
    bf_d = nc.dram_tensor("bfc", [V, 1], F32, kind="ExternalInput")
    bhn_d = nc.dram_tensor("bhn", [H, 1], F32, kind="ExternalInput")
    lo_d = nc.dram_tensor("loT", [V, BS], F32, kind="ExternalOutput")
    h_d = (
        nc.dram_tensor("hT", [H, BS], F32, kind="ExternalOutput")
        if dump_h
        else None
    )

    W = BS // nch
    n_chunks = max(1, t_steps // CHUNK)
    chunk = min(CHUNK, t_steps)

    with tile.TileContext(nc) as tc:
        with (
            tc.tile_pool(name="const", bufs=1) as constp,
            tc.tile_pool(name="state", bufs=1) as statep,
            tc.tile_pool(name="ohp", bufs=2) as ohp,
            tc.tile_pool(name="work", bufs=wbufs) as workp,
            tc.tile_pool(name="psAB", bufs=2, space="PSUM") as psab,
            tc.tile_pool(name="psNG", bufs=2, space="PSUM") as psng,
        ):
            wt = constp.tile([H, 3 * H], F16, tag="wt")
            nc.sync.dma_start(wt[:], wt_d[:])
            gi = constp.tile([V, 3 * H], F16, tag="gi")
            nc.sync.dma_start(gi[:], gi_d[:])
            wf = constp.tile([H, V], F16, tag="wf")
            nc.sync.dma_start(wf[:], wf_d[:])
            bf = constp.tile([V, 1], F32, tag="bf")
            nc.sync.dma_start(bf[:], bf_d[:])
            bhn = constp.tile([H, 1], F32, tag="bhn")
            nc.sync.dma_start(bhn[:], bhn_d[:])
            lo = constp.tile([V, BS], F32, tag="lo")

            h = []
            for c in range(nch):
                hc = statep.tile([H, W], F16, tag=f"h{c}")
                nc.vector.memset(hc[:], 0.0)
                h.append(hc)

            for _rep in range(reps):
              for ck in range(n_chunks):
                oh_t = ohp.tile([V, chunk * BS], F16, tag="oh")
                nc.sync.dma_start(
                    oh_t[:], oh_d[:, ck * chunk * BS : (ck + 1) * chunk * BS]
                )
                for tl in range(chunk):
                    abs_, ngs, rzs, us, ns_, ps, qs = {}, {}, {}, {}, {}, {}, {}
                    for c in range(nch):
                        ohs = oh_t[:, tl * BS + c * W : tl * BS + (c + 1) * W]
                        ab = psab.tile([H, 2 * W], F32, tag=f"ab{c}", name=f"ab{c}")
                        ng = psng.tile([H, 3 * W], F32, tag=f"ng{c}", name=f"ng{c}")
                        abs_[c], ngs[c] = ab, ng

                        # a = gi_r(x_t) + W_r h   (both biases folded into gi)
                        nc.tensor.matmul(
                            ab[:, 0:W], gi[:, 0:H], ohs, start=True, stop=False
                        )
                        nc.tensor.matmul(
                            ab[:, 0:W], wt[:, 0:H], h[c][:], start=False, stop=True
                        )
                        # b = gi_z(x_t) + W_z h
                        nc.tensor.matmul(
                            ab[:, W : 2 * W],
                            gi[:, H : 2 * H],
                            ohs,
                            start=True,
                            stop=False,
                        )
                        nc.tensor.matmul(
                            ab[:, W : 2 * W],
                            wt[:, H : 2 * H],
                            h[c][:],
                            start=False,
                            stop=True,
                        )
                        # ghn = W_n h ; gin = gi_n(x_t)   (kept separate)
                        nc.tensor.matmul(
                            ng[:, 0:W], wt[:, 2 * H : 3 * H], h[c][:],
                            start=True, stop=True,
                        )
                        nc.tensor.matmul(
                            ng[:, W : 2 * W], gi[:, 2 * H : 3 * H], ohs,
                            start=True, stop=True,
                        )

                    for c in range(nch):
                        # r|z = sigmoid(a|b) in one ACT op
                        rz = workp.tile([H, 2 * W], F16, tag=f"rz{c}", name=f"rz{c}")
                        nc.scalar.activation(rz[:], abs_[c][:], AF.Sigmoid)
                        rzs[c] = rz
                    for c in range(nch):
                        # u = r * (ghn + b_hh_n) ; n-input c = u + gin (PSUM)
                        u = workp.tile([H, W], F16, tag=f"u{c}", name=f"u{c}")
                        nc.vector.scalar_tensor_tensor(
                            u[:], ngs[c][:, 0:W], bhn[:], rzs[c][:, 0:W],
                            op0=OP.add, op1=OP.mult,
                        )
                        us[c] = u
                        # p = z*h off the critical path
                        p_t = workp.tile([H, W], F16, tag=f"p{c}", name=f"p{c}")
                        peng = nc.gpsimd if use_gps else nc.vector
                        peng.tensor_mul(p_t[:], rzs[c][:, W : 2 * W], h[c][:])
                        ps[c] = p_t
                    for c in range(nch):
                        nc.vector.tensor_add(
                            ngs[c][:, 2 * W : 3 * W], us[c][:], ngs[c][:, W : 2 * W]
                        )
                    for c in range(nch):
                        n_t = workp.tile([H, W], F16, tag=f"n{c}", name=f"n{c}")
                        nc.scalar.activation(n_t[:], ngs[c][:, 2 * W : 3 * W], AF.Tanh)
                        ns_[c] = n_t
                    for c in range(nch):
                        q_t = workp.tile([H, W], F16, tag=f"q{c}", name=f"q{c}")
                        nc.vector.scalar_tensor_tensor(
                            q_t[:], rzs[c][:, W : 2 * W], 1.0, ns_[c][:],
                            op0=OP.subtract, op1=OP.mult,
                        )
                        nc.vector.tensor_sub(h[c][:], ps[c][:], q_t[:])

            if h_d is not None:
                hd = constp.tile([H, BS], F32, tag="hd")
                for c in range(nch):
                    nc.vector.tensor_copy(hd[:, c * W : (c + 1) * W], h[c][:])
                nc.sync.dma_start(h_d[:], hd[:])

            # logits.T = W_fc @ h + b_fc
            for c in range(nch):
                lg = psab.tile([V, W], F32, tag="ab0")
                nc.tensor.matmul(lg[:], wf[:], h[c][:], start=True, stop=True)
                nc.scalar.activation(
                    lo[:, c * W : (c + 1) * W], lg[:], AF.Identity, bias=bf[:]
                )
            nc.sync.dma_start(lo_d[:], lo[:])

    nc.finalize()
    return nc


_NC_CACHE: dict[tuple, bass.Bass] = {}


def get_nc(t_steps: int = T, reps: int = 1, nch: int = NCH, use_gps: bool = USE_GPS, wbufs: int = WBUFS) -> bass.Bass:
    key = (t_steps, reps, nch, use_gps, wbufs)
    if key not in _NC_CACHE:
        _NC_CACHE[key] = build_nc(t_steps, reps=reps, nch=nch, use_gps=use_gps, wbufs=wbufs)
    return _NC_CACHE[key]


def make_in_maps(x, emb, W_ih, W_hh, b_ih, b_hh, W_fc, b_fc, t_steps: int = T):
    x = np.asarray(x)
    emb = np.asarray(emb, dtype=np.float32)
    W_ih = np.asarray(W_ih, dtype=np.float32)
    W_hh = np.asarray(W_hh, dtype=np.float32)
    b_ih = np.asarray(b_ih, dtype=np.float32)
    b_hh = np.asarray(b_hh, dtype=np.float32)
    W_fc = np.asarray(W_fc, dtype=np.float32)
    b_fc = np.asarray(b_fc, dtype=np.float32)

    # Fold b_ih (all gates) + b_hh (r,z only) into the gi lookup table.
    # b_hh_n must stay inside the reset product: n = tanh(gi_n + r*(W_n h + b_hh_n))
    bias = b_ih.copy()
    bias[: 2 * H] += b_hh[: 2 * H]
    gi_tab = (emb @ W_ih.T + bias).astype(np.float16)  # [V, 3H]
    wt = np.ascontiguousarray(W_hh.T).astype(np.float16)      # [H, 3H]
    wfc = np.ascontiguousarray(W_fc.T).astype(np.float16)     # [H, V]
    bfc = b_fc.reshape(V, 1).astype(np.float32)

    in_maps = []
    for c in range(N_CORES):
        xs = x[c * BS : (c + 1) * BS, :t_steps]               # [BS, t]
        oh = (xs.T[None, :, :] == np.arange(V)[:, None, None]).astype(np.float16)
        in_maps.append(
            {
                "oh": np.ascontiguousarray(oh.reshape(V, t_steps * BS)),
                "WT": wt,
                "giT": gi_tab,
                "WfcT": wfc,
                "bfc": bfc,
                "bhn": b_hh[2 * H :].reshape(H, 1).astype(np.float32),
            }
        )
    return in_maps


def run_cores(in_maps, t_steps: int = T, trace: bool = False, reps: int = 1, nch: int = NCH):
    res = run_bass_kernel_spmd(
        get_nc(t_steps, reps, nch), in_maps, list(range(N_CORES)), trace=trace
    )
    out = np.concatenate([r["loT"].T for r in res.results], axis=0)
    return out.astype(np.float32), res


def kernel(x, emb, W_ih, W_hh, b_ih, b_hh, W_fc, b_fc):
    in_maps = make_in_maps(x, emb, W_ih, W_hh, b_ih, b_hh, W_fc, b_fc)
    out, _ = run_cores(in_maps)
    return out

